# revision 2
# baseline (speedup 1.0000x reference)
"""GNN message-passing (PNA-style) Trainium2 Bass kernel, 8-core SPMD.

Self-contained: hardcodes problem shapes. kernel(**inputs) -> [4000, 1] f32.
"""
import sys
sys.path.insert(0, "/opt/trn_rl_repo")
import numpy as np

import concourse.bass as bass
import concourse.bacc as bacc
import concourse.tile as tile
from concourse import mybir
from concourse.bass_utils import run_bass_kernel_spmd
from concourse.masks import make_identity

fp32 = mybir.dt.float32
i32 = mybir.dt.int32
AF = mybir.ActivationFunctionType
OP = mybir.AluOpType

# problem constants
N, E, G, D = 100000, 400000, 4000, 70
NC = 8
ATOM_DIMS = np.array([119, 5, 12, 12, 10, 6, 6, 2, 2])
ATOM_OFFSETS = np.concatenate([[0], np.cumsum(ATOM_DIMS)[:-1]]).astype(np.int64)
DEG_HIST = np.array([0.0, 100.0, 400.0, 300.0, 200.0])
_bins = np.arange(len(DEG_HIST), dtype=np.float64)
AVG_LOG = float((np.log(_bins + 1.0) * DEG_HIST).sum() / DEG_HIST.sum())
BN_EPS = 1e-5
STD_EPS = 1e-5
P = 128
BT = 4          # tiles per processing block
EA_PAD = -1e9   # ea value for padding slots (message -> 0 after relu)
BIG = 1e9

_COMPILED = None  # cache (nc, cfg-shape signature) within process


def _insert_axis(ap_obj, pos, count):
    """Insert a broadcast axis [step=0, count] at free-dim position pos (0-based
    counting partition dim as index 0 of .ap)."""
    lst = [list(x) for x in ap_obj.ap]
    lst = lst[:pos] + [[0, count]] + lst[pos:]
    return bass.AP(ap_obj.tensor, ap_obj.offset, lst)


def _prep(x, edge_index, edge_attr, batch, atom_emb):
    """Host-side graph preprocessing. Returns cfg + per-core arrays."""
    src = np.asarray(edge_index[0], np.int64)
    dst = np.asarray(edge_index[1], np.int64)
    batch = np.asarray(batch, np.int64)
    ea = np.asarray(edge_attr, np.float32)

    deg = np.bincount(dst, minlength=N)
    # CSR by dst
    eorder = np.argsort(dst, kind="stable")
    rowptr = np.zeros(N + 1, np.int64)
    rowptr[1:] = np.cumsum(deg)

    # graph-aligned core node ranges
    gcnt = np.bincount(batch, minlength=G)
    gnode_start = np.zeros(G + 1, np.int64)
    gnode_start[1:] = np.cumsum(gcnt)
    core_gb = [0]
    for c in range(1, NC):
        target = c * N // NC
        gi = int(np.searchsorted(gnode_start, target))
        if gnode_start[gi] != target and gi > 0:
            gi = gi if abs(gnode_start[gi] - target) < abs(gnode_start[gi - 1] - target) else gi - 1
        core_gb.append(gi)
    core_gb.append(G)
    core_nodes = [(int(gnode_start[core_gb[c]]), int(gnode_start[core_gb[c + 1]])) for c in range(NC)]

    # degree groups: exact 1..8, tail >= 9 (padded to dtail)
    dmax = int(deg.max())
    exact_ds = [d for d in range(0, min(dmax, 8) + 1)]
    has_tail = dmax > 8
    dtail = dmax if has_tail else 0

    # per-core nodes per group
    core_group_nodes = []  # [c][g] -> node id array
    for c in range(NC):
        n0, n1 = core_nodes[c]
        nd = deg[n0:n1]
        groups = []
        for d in exact_ds:
            groups.append(np.nonzero(nd == d)[0] + n0)
        if has_tail:
            groups.append(np.nonzero(nd >= 9)[0] + n0)
        core_group_nodes.append(groups)

    ngroups = len(exact_ds) + (1 if has_tail else 0)
    dvals = exact_ds + ([dtail] if has_tail else [])
    NT_g = [max((len(core_group_nodes[c][g]) + P - 1) // P for c in range(NC)) for g in range(ngroups)]
    NT = sum(NT_g)
    NB = NT * P
    NPAD = NC * NB

    # proc order per core: group-major, node arrays padded with -1
    proc = np.full((NC, NB), -1, np.int64)
    tile_d = np.zeros(NT, np.int64)  # degree of each tile (common)
    ti = 0
    goff = []
    for g in range(ngroups):
        goff.append(ti)
        for t in range(NT_g[g]):
            tile_d[ti] = dvals[g]
            ti += 1
    for c in range(NC):
        for g in range(ngroups):
            nodes = core_group_nodes[c][g]
            off = goff[g] * P
            proc[c, off:off + len(nodes)] = nodes

    # pos_of_node (per core, global padded position)
    gpos_of_node = np.full(N, -1, np.int64)
    for c in range(NC):
        mask = proc[c] >= 0
        gpos_of_node[proc[c][mask]] = c * NB + np.nonzero(mask)[0]
    assert (gpos_of_node >= 0).sum() == N

    # blocks: per group, tiles chunked by BT
    blocks = []  # (g, d, t0, nb) with t0 global tile index
    for g in range(ngroups):
        d = dvals[g]
        for b0 in range(0, NT_g[g], BT):
            nb = min(BT, NT_g[g] - b0)
            blocks.append((g, d, goff[g] + b0, nb))

    # gather columns + ea layout + bigpad
    ncols = sum(d * nb for (_, d, _, nb) in blocks if d > 0)
    srcidx = np.full((NC, P, max(ncols, 1)), NPAD, np.int32)
    srcidx2 = np.full((NC, P, max(ncols, 1)), NPAD, np.int32)
    ea_parts = {}  # c -> list of [P, X] blocks
    for c in range(NC):
        ea_parts[c] = []
    bigpad_cols = sum(dtail * nb for (g, d, _, nb) in blocks if has_tail and g == ngroups - 1)
    bigpad = np.zeros((NC, P, max(bigpad_cols, 1)), np.float32)

    col = 0
    bcol = 0
    ea_offsets = []  # block -> offset in flat ea (same all cores)
    ea_off = 0
    for (g, d, t0, nb) in blocks:
        if d == 0:
            continue
        ea_offsets.append((g, d, t0, nb, col, ea_off, bcol))
        for c in range(NC):
            blk = np.full((P, nb * d, D), EA_PAD, np.float32)
            for i in range(nb):
                t = t0 + i
                for p in range(P):
                    node = proc[c, t * P + p]
                    if node < 0:
                        continue
                    nd = int(deg[node])
                    use = min(nd, d)
                    eids = eorder[rowptr[node]:rowptr[node] + use]
                    blk[p, i * d:i * d + use] = ea[eids]
                    srcidx[c, p, col + i * d:col + i * d + use] = gpos_of_node[src[eids]]
                    if g == ngroups - 1 and has_tail and nd < d:
                        bigpad[c, p, bcol + i * d + nd:bcol + (i + 1) * d] = BIG
            ea_parts[c].append(blk.reshape(P, -1))
        col += nb * d
        ea_off += P * nb * d * D
        if g == ngroups - 1 and has_tail:
            bcol += nb * d
    NIDX = col
    eaflat = np.stack([np.concatenate([b.ravel() for b in ea_parts[c]]) if ea_parts[c] else np.zeros(1, np.float32) for c in range(NC)])
    TOTEA = eaflat.shape[1]

    # split point for chunked allgather: block boundary at ~55% of gather cols
    cum = 0
    Tsplit = ea_offsets[-1][2] + ea_offsets[-1][3] if ea_offsets else NT
    for (g, d, t0, nb, bcol_, eoff_, bp_) in ea_offsets:
        cum += nb * d
        if cum >= 0.72 * NIDX:
            Tsplit = t0 + nb
            break
    H = Tsplit * P
    # gpos2: split-allgather layout: pos<H -> c*H+pos ; else 8H + c*(NB-H) + (pos-H)
    v = srcidx.astype(np.int64)
    cc_ = v // NB
    pp_ = v - cc_ * NB
    lo = v < NPAD
    g2 = np.where(pp_ < H, cc_ * H + pp_, NC * H + cc_ * (NB - H) + (pp_ - H))
    srcidx2 = np.where(lo, g2, NPAD).astype(np.int32)

    # statics in proc order [NC, 3, NT, P]
    statn = np.zeros((NC, 3, NT, P), np.float32)
    for c in range(NC):
        nodes = proc[c]
        dd = np.where(nodes >= 0, deg[np.clip(nodes, 0, N - 1)], 0).astype(np.float64)
        dsafe = np.maximum(dd, 1.0)
        amp = np.log(dsafe + 1.0) / AVG_LOG
        statn[c, 0] = (1.0 / dsafe).astype(np.float32).reshape(NT, P)
        statn[c, 1] = amp.astype(np.float32).reshape(NT, P)
        statn[c, 2] = (1.0 / amp).astype(np.float32).reshape(NT, P)

    # h0 (AtomEncoder) host-side, proc order
    xl = np.asarray(x, np.int64)
    emb = np.asarray(atom_emb, np.float32)
    h0_all = emb[xl + ATOM_OFFSETS[None, :]].sum(axis=1)  # [N, D]
    h0_own = np.zeros((NC, NB, D), np.float32)
    for c in range(NC):
        mask = proc[c] >= 0
        h0_own[c][mask] = h0_all[proc[c][mask]]

    # pooling: per core graphs sorted by size desc, common tile grid
    core_graphs = []
    for c in range(NC):
        gids = np.arange(core_gb[c], core_gb[c + 1])
        order = np.argsort(-gcnt[gids], kind="stable")
        core_graphs.append(gids[order])
    NGT = max((len(cg) + P - 1) // P for cg in core_graphs)
    KG_t = []
    for t in range(NGT):
        m = 1
        for c in range(NC):
            cg = core_graphs[c]
            if t * P < len(cg):
                m = max(m, int(gcnt[cg[t * P]]))
        KG_t.append(m)
    npoolcols = sum(KG_t)
    poolidx = np.full((NC, P, npoolcols), NB, np.int32)
    ginv = np.ones((NC, NGT, P), np.float32)
    pc = 0
    pool_cols = []
    for t in range(NGT):
        pool_cols.append(pc)
        for c in range(NC):
            cg = core_graphs[c]
            for p in range(P):
                if t * P + p >= len(cg):
                    continue
                gid = cg[t * P + p]
                sz = int(gcnt[gid])
                ginv[c, t, p] = 1.0 / max(sz, 1)
                if sz > 0:
                    nids = np.arange(gnode_start[gid], gnode_start[gid] + sz)
                    pos = gpos_of_node[nids] - (gpos_of_node[nids] // NB) * NB
                    # nodes of this graph belong to core c by construction
                    poolidx[c, p, pc:pc + sz] = pos.astype(np.int32)
        pc += KG_t[t]

    cfg = dict(NB=NB, NT=NT, NPAD=NPAD, NIDX=NIDX, TOTEA=TOTEA, Tsplit=Tsplit, H=H,
               blocks=blocks, ea_offsets=ea_offsets, dvals=dvals, goff=goff,
               NT_g=NT_g, ngroups=ngroups, has_tail=has_tail, dtail=dtail,
               NGT=NGT, KG_t=KG_t, pool_cols=pool_cols, npoolcols=npoolcols,
               bigpad_cols=max(bigpad_cols, 1))
    h0_full = np.zeros((NPAD + 1, D), np.float32)
    for c in range(NC):
        h0_full[c * NB:(c + 1) * NB] = h0_own[c]
    arrays = dict(srcidx=srcidx, srcidx2=srcidx2, eaflat=eaflat, statn=statn,
                  h0_own=h0_own, h0_full=h0_full, poolidx=poolidx, ginv=ginv, bigpad=bigpad)
    asm = dict(core_graphs=core_graphs, core_gb=core_gb)
    return cfg, arrays, asm


def _prep_weights(post_w, post_b, bn_gamma, bn_beta, mlp_w1, mlp_b1, mlp_w2, mlp_b2, mlp_w3, mlp_b3):
    post_w = np.asarray(post_w, np.float32)   # [4, 840, 70]
    post_b = np.asarray(post_b, np.float32)   # [4, 70]
    bn_gamma = np.asarray(bn_gamma, np.float32)
    bn_beta = np.asarray(bn_beta, np.float32)
    inv_std_bn = np.float32(1.0 / np.sqrt(1.0 + BN_EPS))
    # wch [4, 3 chunks, 128, 210]: chunk k rows k*128..; cols = A|B|C (70 each)
    wch = np.zeros((4, 3, P, 210), np.float32)
    for l in range(4):
        for ch in range(3):
            r0, r1 = ch * 128, min((ch + 1) * 128, 280)
            rows = r1 - r0
            for s in range(3):  # A, B, C weight sets at rows s*280
                wch[l, ch, :rows, s * 70:(s + 1) * 70] = post_w[l, s * 280 + r0:s * 280 + r1, :]
    Grep = (bn_gamma * inv_std_bn)                     # [4, 70]
    B2 = post_b * Grep + bn_beta                       # [4, 70]
    w1 = np.asarray(mlp_w1, np.float32)                # [70, 35]
    w2 = np.asarray(mlp_w2, np.float32)                # [35, 17]
    w3 = np.asarray(mlp_w3, np.float32)                # [17, 1]
    b1 = np.asarray(mlp_b1, np.float32)
    b2 = np.asarray(mlp_b2, np.float32)
    b3 = np.asarray(mlp_b3, np.float32)
    # reps [128, 4*70 + 4*70 + 35 + 17 + 1]
    reps = np.concatenate([Grep.ravel(), B2.ravel(), b1, b2, b3]).astype(np.float32)
    reps = np.broadcast_to(reps, (P, reps.size)).copy()
    return dict(wch=wch, reps=reps, w1=w1, w2=w2, w3=w3)


def _build(cfg):
    NB, NT, NPAD, NIDX, TOTEA = cfg["NB"], cfg["NT"], cfg["NPAD"], cfg["NIDX"], cfg["TOTEA"]
    NGT, npoolcols = cfg["NGT"], cfg["npoolcols"]
    NREP = 4 * 70 + 4 * 70 + 35 + 17 + 1

    nc = bacc.Bacc("TRN2", target_bir_lowering=False, debug=False, num_devices=NC)
    # inputs
    h0_own = nc.dram_tensor("h0_own", [NB, D], fp32, kind="ExternalInput").ap()
    h0_full = nc.dram_tensor("h0_full", [NPAD + 1, D], fp32, kind="ExternalInput").ap()
    srcidx2 = nc.dram_tensor("srcidx2", [P, max(NIDX, 1)], i32, kind="ExternalInput").ap()
    eaflat = nc.dram_tensor("eaflat", [TOTEA], fp32, kind="ExternalInput").ap()
    srcidx = nc.dram_tensor("srcidx", [P, max(NIDX, 1)], i32, kind="ExternalInput").ap()
    statn = nc.dram_tensor("statn", [3, NT, P], fp32, kind="ExternalInput").ap()
    bigpad_t = nc.dram_tensor("bigpad", [P, cfg["bigpad_cols"]], fp32, kind="ExternalInput").ap()
    poolidx = nc.dram_tensor("poolidx", [P, npoolcols], i32, kind="ExternalInput").ap()
    ginv = nc.dram_tensor("ginv", [NGT, P], fp32, kind="ExternalInput").ap()
    wch = nc.dram_tensor("wch", [4, 3, P, 210], fp32, kind="ExternalInput").ap()
    reps = nc.dram_tensor("reps", [P, NREP], fp32, kind="ExternalInput").ap()
    w1 = nc.dram_tensor("w1", [D, 35], fp32, kind="ExternalInput").ap()
    w2 = nc.dram_tensor("w2", [35, 17], fp32, kind="ExternalInput").ap()
    w3 = nc.dram_tensor("w3", [17, 1], fp32, kind="ExternalInput").ap()
    out_g = nc.dram_tensor("out_g", [NGT * P, 1], fp32, kind="ExternalOutput").ap()

    # internal DRAM
    h_own = [nc.dram_tensor(f"h_own{l}", [NB + 1, D], fp32) for l in range(5)]
    hbuf = [None] + [nc.dram_tensor(f"hbuf{l}", [NPAD + 1, D], fp32) for l in range(1, 4)]

    # persistent SBUF
    idx_sb = nc.alloc_sbuf_tensor("idx_sb", [P, max(NIDX, 1)], i32).ap()
    idx2_sb = nc.alloc_sbuf_tensor("idx2_sb", [P, max(NIDX, 1)], i32).ap()
    pidx_sb = nc.alloc_sbuf_tensor("pidx_sb", [P, npoolcols], i32).ap()
    wch_sb = nc.alloc_sbuf_tensor("wch_sb", [P, 4 * 3 * 210], fp32).ap()
    reps_sb = nc.alloc_sbuf_tensor("reps_sb", [P, NREP], fp32).ap()
    w1_sb = nc.alloc_sbuf_tensor("w1_sb", [D, 35], fp32).ap()
    w2_sb = nc.alloc_sbuf_tensor("w2_sb", [35, 17], fp32).ap()
    w3_sb = nc.alloc_sbuf_tensor("w3_sb", [17, 1], fp32).ap()
    ident = nc.alloc_sbuf_tensor("ident", [P, P], fp32).ap()
    epsb = nc.alloc_sbuf_tensor("epsb", [P, 1], fp32).ap()
    zrow = nc.alloc_sbuf_tensor("zrow", [1, D], fp32).ap()

    cc_sems = {(l, h): nc.alloc_semaphore(name=f"ccs{l}_{h}") for l in range(1, 4) for h in range(2)}

    # ---- segment 0: load persistents, init dummies, stage h0 ----
    with tile.TileContext(nc) as tc:
        with tc.tile_pool(name="s0", bufs=2) as pool:
            nc.sync.dma_start(out=idx_sb[:, :], in_=srcidx[:, :])
            nc.sync.dma_start(out=idx2_sb[:, :], in_=srcidx2[:, :])
            nc.sync.dma_start(out=pidx_sb[:, :], in_=poolidx[:, :])
            nc.sync.dma_start(out=wch_sb[:].rearrange("p (l c f) -> p l c f", l=4, c=3),
                              in_=wch.rearrange("l c p f -> p l c f"))
            nc.sync.dma_start(out=reps_sb[:, :], in_=reps[:, :])
            nc.sync.dma_start(out=w1_sb[:, :], in_=w1[:, :])
            nc.sync.dma_start(out=w2_sb[:, :], in_=w2[:, :])
            nc.sync.dma_start(out=w3_sb[:, :], in_=w3[:, :])
            make_identity(nc, ident[:])
            nc.vector.memset(epsb[:], STD_EPS)
            nc.vector.memset(zrow[:], 0.0)
            for l in range(1, 4):
                nc.sync.dma_start(out=hbuf[l].ap()[NPAD:NPAD + 1, :], in_=zrow[:])
            nc.sync.dma_start(out=h_own[4].ap()[NB:NB + 1, :], in_=zrow[:])

    H = cfg["H"]
    Tsplit = cfg["Tsplit"]

    def do_cc(l, part):
        if part == 0:
            ins_ap = h_own[l].ap()[0:H, :].opt()
            outs_ap = hbuf[l].ap()[0:NC * H, :].opt()
        else:
            ins_ap = h_own[l].ap()[H:NB, :].opt()
            outs_ap = hbuf[l].ap()[NC * H:NPAD, :].opt()
        nc.gpsimd.collective_compute(
            "AllGather", OP.bypass,
            replica_groups=[list(range(NC))],
            ins=[ins_ap], outs=[outs_ap],
        ).then_inc(cc_sems[(l, part)])

    def emit_msg_block(blk, l, hprev_own, hprev_full, idxtile, pool, spool, psp, gpool):
        (g, d, t0, nb, col, ea_off, bcol) = blk
        X = nb * d * D
        gblk = gpool.tile([P, X], fp32, tag="gblk")
        gsrc = gpool.tile([P, X], fp32, tag="gsrc")
        nc.sync.dma_start(out=gblk[:], in_=eaflat[ea_off:ea_off + P * X].rearrange("(p x) -> p x", p=P))
        for i in range(nb):
            for j in range(d):
                cidx = col + i * d + j
                nc.gpsimd.indirect_dma_start(
                    out=gsrc[:, (i * d + j) * D:(i * d + j + 1) * D],
                    out_offset=None,
                    in_=hprev_full[:, :],
                    in_offset=bass.IndirectOffsetOnAxis(ap=idxtile[:, cidx:cidx + 1], axis=0),
                )
        hdst = pool.tile([P, nb * D], fp32, tag="hdst")
        nc.sync.dma_start(
            out=hdst[:],
            in_=hprev_own[t0 * P:(t0 + nb) * P, :].rearrange("(t p) f -> p t f", p=P))
        nc.vector.tensor_tensor(out=gblk[:], in0=gblk[:], in1=gsrc[:], op=OP.add)
        hdst_b = _insert_axis(hdst[:].rearrange("p (t f) -> p t f", t=nb), 2, d)
        g3 = gblk[:].rearrange("p (t j f) -> p t j f", t=nb, j=d)
        nc.vector.tensor_tensor(out=g3, in0=g3, in1=hdst_b, op=OP.add)
        nc.scalar.activation(out=gblk[:], in_=gblk[:], func=AF.Relu)

        agg = spool.tile([P, nb * 280], fp32, tag="agg")
        a3 = agg[:].rearrange("p (t f) -> p t f", t=nb)
        mn_out = a3[:, :, 70:140]
        mx_out = a3[:, :, 140:210]
        is_tail = cfg["has_tail"] and g == cfg["ngroups"] - 1
        if d == 1:
            nc.vector.tensor_copy(out=mn_out, in_=gblk[:].rearrange("p (t f) -> p t f", t=nb))
            nc.vector.tensor_copy(out=mx_out, in_=gblk[:].rearrange("p (t f) -> p t f", t=nb))
            s_src = gblk[:].rearrange("p (t f) -> p t f", t=nb)
            nc.vector.tensor_copy(out=a3[:, :, 0:70], in_=s_src)
            nc.scalar.activation(out=gblk[:], in_=gblk[:], func=AF.Square)
            s2_fin = gblk[:].rearrange("p (t f) -> p t f", t=nb)
            s_fin = a3[:, :, 0:70]
        else:
            g4 = gblk[:].rearrange("p (t j f) -> p t j f", t=nb, j=d)
            if is_tail:
                bp = pool.tile([P, nb * d], fp32, tag="bp")
                nc.sync.dma_start(out=bp[:], in_=bigpad_t[:, bcol:bcol + nb * d])
                mfm = spool.tile([P, X], fp32, tag="mfm")
                m4 = mfm[:].rearrange("p (t j f) -> p t j f", t=nb, j=d)
                bp_b = _insert_axis(bp[:].rearrange("p (t j) -> p t j", t=nb), 3, D)
                nc.vector.tensor_tensor(out=m4, in0=g4, in1=bp_b, op=OP.add)
                _fold_minmax(nc, spool, m4, d, nb, mn_out, OP.min, "mnscr")
            else:
                _fold_minmax(nc, spool, g4, d, nb, mn_out, OP.min, "mnscr")
            _fold_minmax(nc, spool, g4, d, nb, mx_out, OP.max, "mxscr")
            _fold_sum(nc, spool, g4, d, nb, a3[:, :, 0:70], "sscr")
            nc.scalar.activation(out=gblk[:], in_=gblk[:], func=AF.Square)
            s2t = spool.tile([P, nb * D], fp32, tag="s2t")
            _fold_sum(nc, spool, g4, d, nb, s2t[:].rearrange("p (t f) -> p t f", t=nb), "s2scr")
            s2_fin = s2t[:].rearrange("p (t f) -> p t f", t=nb)
            s_fin = a3[:, :, 0:70]
        _stage2(nc, pool, spool, psp, cfg, statn, a3, s_fin, s2_fin,
                t0, nb, l, wch_sb, reps_sb, ident, epsb, hprev_own, h_own[l].ap(), d)

    def emit_d0_block(blk, l, hprev_own, pool, spool, psp):
        (g, d, t0, nb) = blk
        agg = spool.tile([P, nb * 280], fp32, tag="agg")
        nc.vector.memset(agg[:], 0.0)
        a3 = agg[:].rearrange("p (t f) -> p t f", t=nb)
        _stage2(nc, pool, spool, psp, cfg, statn, a3, a3[:, :, 0:70], a3[:, :, 0:70],
                t0, nb, l, wch_sb, reps_sb, ident, epsb, hprev_own, h_own[l].ap(), d)

    easA = [b for b in cfg["ea_offsets"] if b[2] < Tsplit]
    easB = [b for b in cfg["ea_offsets"] if b[2] >= Tsplit]
    d0sA = [b for b in cfg["blocks"] if b[1] == 0 and b[2] < Tsplit]
    d0sB = [b for b in cfg["blocks"] if b[1] == 0 and b[2] >= Tsplit]
    for b in easA + [x + (0, 0, 0) for x in d0sA]:
        assert b[2] + b[3] <= Tsplit, b

    # ---- layers ----
    for l in range(1, 5):
        hprev_own = h0_own if l == 1 else h_own[l - 1].ap()
        hprev_full = h0_full if l == 1 else hbuf[l - 1].ap()
        idxtile = idx_sb if l == 1 else idx2_sb
        if l >= 2 and l - 1 <= 3:
            nc.gpsimd.wait_ge(cc_sems[(l - 1, 0)], 1)
            nc.gpsimd.wait_ge(cc_sems[(l - 1, 1)], 1)
        if l < 4:
            with tile.TileContext(nc) as tc:
                with tc.tile_pool(name=f"L{l}a", bufs=2) as pool, \
                     tc.tile_pool(name=f"Lg{l}a", bufs=3) as gpool, \
                     tc.tile_pool(name=f"Ls{l}a", bufs=1) as spool, \
                     tc.tile_pool(name=f"Lp{l}a", bufs=2, space="PSUM") as psp:
                    for blk in easA:
                        emit_msg_block(blk, l, hprev_own, hprev_full, idxtile, pool, spool, psp, gpool)
                    for blk in d0sA:
                        emit_d0_block(blk, l, hprev_own, pool, spool, psp)
            do_cc(l, 0)
            with tile.TileContext(nc) as tc:
                with tc.tile_pool(name=f"L{l}b", bufs=2) as pool, \
                     tc.tile_pool(name=f"Lg{l}b", bufs=3) as gpool, \
                     tc.tile_pool(name=f"Ls{l}b", bufs=1) as spool, \
                     tc.tile_pool(name=f"Lp{l}b", bufs=2, space="PSUM") as psp:
                    for blk in easB:
                        emit_msg_block(blk, l, hprev_own, hprev_full, idxtile, pool, spool, psp, gpool)
                    for blk in d0sB:
                        emit_d0_block(blk, l, hprev_own, pool, spool, psp)
            do_cc(l, 1)
        else:
            with tile.TileContext(nc) as tc:
                with tc.tile_pool(name=f"L{l}", bufs=2) as pool, \
                     tc.tile_pool(name=f"Lg{l}", bufs=3) as gpool, \
                     tc.tile_pool(name=f"Ls{l}", bufs=1) as spool, \
                     tc.tile_pool(name=f"Lp{l}", bufs=2, space="PSUM") as psp:
                    for blk in easA + easB:
                        emit_msg_block(blk, l, hprev_own, hprev_full, idxtile, pool, spool, psp, gpool)
                    for blk in d0sA + d0sB:
                        emit_d0_block(blk, l, hprev_own, pool, spool, psp)
                    _pooling(nc, pool, spool, psp, cfg, pidx_sb, ginv, h_own[4].ap(),
                             w1_sb, w2_sb, w3_sb, reps_sb, ident, out_g)

    nc.compile()
    return nc


def _fold_minmax(nc, spool, g4, d, nb, out_slice, op, tag):
    k = d
    cur = g4
    first = True
    while k > 1:
        h = (k + 1) // 2
        if k == 2:
            nc.vector.tensor_tensor(
                out=out_slice,
                in0=cur[:, :, 0:1].rearrange("p t j f -> p t (j f)"),
                in1=cur[:, :, 1:2].rearrange("p t j f -> p t (j f)"), op=op)
            return
        if first:
            scr = spool.tile([g4.shape[0], nb * h * 70], fp32, tag=tag)
            scr3 = scr[:].rearrange("p (t j f) -> p t j f", t=nb, j=h)
            nc.vector.tensor_tensor(out=scr3[:, :, 0:h], in0=cur[:, :, 0:h], in1=cur[:, :, k - h:k], op=op)
            cur = scr3
            first = False
        else:
            nc.vector.tensor_tensor(out=cur[:, :, 0:h], in0=cur[:, :, 0:h], in1=cur[:, :, k - h:k], op=op)
        k = h
    # k == 1 from the start (d==1 handled by caller)


def _fold_sum(nc, spool, g4, d, nb, out_slice, tag):
    """sum over j; out_slice [P, nb, 70]."""
    k = d
    cur = g4
    first = True
    while k > 1:
        h = k // 2
        rem = k - h
        if k == 2:
            nc.vector.tensor_tensor(
                out=out_slice,
                in0=cur[:, :, 0:1].rearrange("p t j f -> p t (j f)"),
                in1=cur[:, :, 1:2].rearrange("p t j f -> p t (j f)"), op=OP.add)
            return
        if first:
            scr = spool.tile([g4.shape[0], nb * rem * 70], fp32, tag=tag)
            scr3 = scr[:].rearrange("p (t j f) -> p t j f", t=nb, j=rem)
            nc.vector.tensor_tensor(out=scr3[:, :, 0:h], in0=cur[:, :, 0:h], in1=cur[:, :, k - h:k], op=OP.add)
            if k % 2 == 1:
                nc.vector.tensor_copy(out=scr3[:, :, h:h + 1], in_=cur[:, :, h:h + 1])
            cur = scr3
            first = False
        else:
            nc.vector.tensor_tensor(out=cur[:, :, 0:h], in0=cur[:, :, 0:h], in1=cur[:, :, k - h:k], op=OP.add)
            # middle element (if odd) already at position h, stays
        k = rem


def _stage2(nc, pool, spool, psp, cfg, statn, a3, s_fin, s2_fin,
            t0, nb, l, wch_sb, reps_sb, ident, epsb, hprev_own, hout, d):
    P_ = 128
    fp = fp32
    # stat tiles
    st = pool.tile([P_, 3 * nb], fp, tag="st")
    for k in range(3):
        nc.sync.dma_start(out=st[:, k * nb:(k + 1) * nb],
                          in_=statn[k, t0:t0 + nb, :].rearrange("t p -> p t"))
    invc_b = _insert_axis(st[:, 0:nb], 2, 70)
    amp_b = _insert_axis(st[:, nb:2 * nb], 2, 70)
    iamp_b = _insert_axis(st[:, 2 * nb:3 * nb], 2, 70)

    if d > 0:
        # mean
        nc.vector.tensor_tensor(out=a3[:, :, 0:70], in0=s_fin, in1=invc_b, op=OP.mult)
        # var/std
        u = spool.tile([P_, nb * 70], fp, tag="u")
        u3 = u[:].rearrange("p (t f) -> p t f", t=nb)
        nc.vector.tensor_tensor(out=u3, in0=s2_fin, in1=invc_b, op=OP.mult)
        v = spool.tile([P_, nb * 70], fp, tag="v")
        v3 = v[:].rearrange("p (t f) -> p t f", t=nb)
        nc.vector.tensor_tensor(out=v3, in0=a3[:, :, 0:70], in1=a3[:, :, 0:70], op=OP.mult)
        nc.vector.tensor_tensor(out=u3, in0=u3, in1=v3, op=OP.subtract)
        nc.scalar.activation(out=u[:], in_=u[:], func=AF.Relu)
        nc.scalar.activation(out=a3[:, :, 210:280], in_=u3, func=AF.Sqrt, bias=epsb[:])
    else:
        # all-zero aggregates; std = sqrt(eps)
        nc.scalar.activation(out=a3[:, :, 210:280], in_=a3[:, :, 0:70], func=AF.Sqrt, bias=epsb[:])

    # post matmul per tile
    sabc = spool.tile([P_, nb * 210], fp, tag="sabc")
    for i in range(nb):
        aggT = pool.tile([P_, P_], fp, tag="aggT")
        psmm = psp.tile([P_, 210], fp, space="PSUM", tag="psmm")
        for ch in range(3):
            rows = 128 if ch < 2 else 24
            psT = psp.tile([P_, P_], fp, space="PSUM", tag="psT")
            nc.tensor.transpose(out=psT[:rows, :], in_=a3[:, i:i + 1, ch * 128:ch * 128 + rows].rearrange("p t f -> p (t f)"),
                                identity=ident[:])
            nc.vector.tensor_copy(out=aggT[:rows, :], in_=psT[:rows, :])
            nc.tensor.matmul(out=psmm[:, :], lhsT=aggT[:rows, :],
                             rhs=wch_sb[:rows, (l - 1) * 630 + ch * 210:(l - 1) * 630 + (ch + 1) * 210],
                             start=(ch == 0), stop=(ch == 2))
        nc.vector.tensor_copy(out=sabc[:, i * 210:(i + 1) * 210], in_=psmm[:, :])

    sA = sabc[:].rearrange("p (t f) -> p t f", t=nb)[:, :, 0:70]
    sB = sabc[:].rearrange("p (t f) -> p t f", t=nb)[:, :, 70:140]
    sC = sabc[:].rearrange("p (t f) -> p t f", t=nb)[:, :, 140:210]
    hn = pool.tile([P_, nb * 70], fp, tag="hn")
    hn3 = hn[:].rearrange("p (t f) -> p t f", t=nb)
    tmp = pool.tile([P_, nb * 70], fp, tag="tmp")
    tmp3 = tmp[:].rearrange("p (t f) -> p t f", t=nb)
    nc.vector.tensor_tensor(out=hn3, in0=sB, in1=amp_b, op=OP.mult)
    nc.vector.tensor_tensor(out=tmp3, in0=sC, in1=iamp_b, op=OP.mult)
    nc.vector.tensor_tensor(out=hn3, in0=hn3, in1=tmp3, op=OP.add)
    nc.vector.tensor_tensor(out=hn3, in0=hn3, in1=sA, op=OP.add)
    # BN affine + relu
    Grep_b = _insert_axis(reps_sb[:, (l - 1) * 70:l * 70], 1, nb)
    B2_b = _insert_axis(reps_sb[:, 280 + (l - 1) * 70:280 + l * 70], 1, nb)
    nc.vector.tensor_tensor(out=hn3, in0=hn3, in1=Grep_b, op=OP.mult)
    nc.vector.tensor_tensor(out=hn3, in0=hn3, in1=B2_b, op=OP.add)
    nc.scalar.activation(out=hn[:], in_=hn[:], func=AF.Relu)
    # residual
    rb = pool.tile([P_, nb * 70], fp, tag="rb")
    nc.sync.dma_start(out=rb[:], in_=hprev_own[t0 * P_:(t0 + nb) * P_, :].rearrange("(t p) f -> p t f", p=P_))
    nc.vector.tensor_tensor(out=hn[:], in0=hn[:], in1=rb[:], op=OP.add)
    nc.sync.dma_start(out=hout[t0 * P_:(t0 + nb) * P_, :].rearrange("(t p) f -> p t f", p=P_),
                      in_=hn[:].rearrange("p (t f) -> p t f", t=nb))


def _pooling(nc, pool, spool, psp, cfg, pidx_sb, ginv, h4, w1_sb, w2_sb, w3_sb, reps_sb, ident, out_g):
    P_ = 128
    fp = fp32
    boff = 560
    for t in range(cfg["NGT"]):
        KG = cfg["KG_t"][t]
        pc = cfg["pool_cols"][t]
        pg = pool.tile([P_, KG * D], fp, tag="pg")
        for j in range(KG):
            nc.gpsimd.indirect_dma_start(
                out=pg[:, j * D:(j + 1) * D], out_offset=None,
                in_=h4[:, :],
                in_offset=bass.IndirectOffsetOnAxis(ap=pidx_sb[:, pc + j:pc + j + 1], axis=0))
        gsum = pool.tile([P_, D], fp, tag="gsum")
        nc.vector.tensor_reduce(out=gsum[:], in_=pg[:].rearrange("p (k f) -> p f k", k=KG),
                                op=OP.add, axis=mybir.AxisListType.X)
        gv = pool.tile([P_, 1], fp, tag="gv")
        nc.sync.dma_start(out=gv[:], in_=ginv[t:t + 1, :].rearrange("o p -> p o"))
        nc.vector.tensor_scalar_mul(gsum[:], gsum[:], gv[:])
        # mlp
        psT = psp.tile([P_, P_], fp, space="PSUM", tag="psT")
        nc.tensor.transpose(out=psT[:D, :], in_=gsum[:], identity=ident[:])
        gT = pool.tile([D, P_], fp, tag="gT")
        nc.vector.tensor_copy(out=gT[:], in_=psT[:D, :])
        ps1 = psp.tile([P_, 35], fp, space="PSUM", tag="psmm")
        nc.tensor.matmul(out=ps1[:], lhsT=gT[:], rhs=w1_sb[:, :], start=True, stop=True)
        y1 = pool.tile([P_, 35], fp, tag="y1")
        nc.vector.tensor_tensor(out=y1[:], in0=ps1[:], in1=reps_sb[:, boff:boff + 35], op=OP.add)
        nc.scalar.activation(out=y1[:], in_=y1[:], func=AF.Relu)
        psT2 = psp.tile([P_, P_], fp, space="PSUM", tag="psT")
        nc.tensor.transpose(out=psT2[:35, :], in_=y1[:], identity=ident[:])
        y1T = pool.tile([35, P_], fp, tag="y1T")
        nc.vector.tensor_copy(out=y1T[:], in_=psT2[:35, :])
        ps2 = psp.tile([P_, 17], fp, space="PSUM", tag="psmm")
        nc.tensor.matmul(out=ps2[:], lhsT=y1T[:], rhs=w2_sb[:, :], start=True, stop=True)
        y2 = pool.tile([P_, 17], fp, tag="y2")
        nc.vector.tensor_tensor(out=y2[:], in0=ps2[:], in1=reps_sb[:, boff + 35:boff + 52], op=OP.add)
        nc.scalar.activation(out=y2[:], in_=y2[:], func=AF.Relu)
        psT3 = psp.tile([P_, P_], fp, space="PSUM", tag="psT")
        nc.tensor.transpose(out=psT3[:17, :], in_=y2[:], identity=ident[:])
        y2T = pool.tile([17, P_], fp, tag="y2T")
        nc.vector.tensor_copy(out=y2T[:], in_=psT3[:17, :])
        ps3 = psp.tile([P_, 1], fp, space="PSUM", tag="psmm")
        nc.tensor.matmul(out=ps3[:], lhsT=y2T[:], rhs=w3_sb[:, :], start=True, stop=True)
        y3 = pool.tile([P_, 1], fp, tag="y3")
        nc.vector.tensor_tensor(out=y3[:], in0=ps3[:], in1=reps_sb[:, boff + 52:boff + 53], op=OP.add)
        nc.sync.dma_start(out=out_g[t * P_:(t + 1) * P_, :], in_=y3[:])


def kernel(x, edge_index, edge_attr, batch, atom_emb, post_w, post_b,
           bn_gamma, bn_beta, mlp_w1, mlp_b1, mlp_w2, mlp_b2, mlp_w3, mlp_b3):
    cfg, arrays, asm = _prep(x, edge_index, edge_attr, batch, atom_emb)
    wd = _prep_weights(post_w, post_b, bn_gamma, bn_beta, mlp_w1, mlp_b1,
                       mlp_w2, mlp_b2, mlp_w3, mlp_b3)
    nc = _build(cfg)

    in_maps = []
    for c in range(NC):
        in_maps.append({
            "h0_own": arrays["h0_own"][c],
            "eaflat": arrays["eaflat"][c].astype(np.float32),
            "srcidx": arrays["srcidx"][c],
            "srcidx2": arrays["srcidx2"][c],
            "h0_full": arrays["h0_full"],
            "statn": arrays["statn"][c],
            "bigpad": arrays["bigpad"][c],
            "poolidx": arrays["poolidx"][c],
            "ginv": arrays["ginv"][c],
            "wch": wd["wch"],
            "reps": wd["reps"],
            "w1": wd["w1"],
            "w2": wd["w2"],
            "w3": wd["w3"],
        })
    import os
    trace = os.environ.get("KERNEL_TRACE", "0") == "1"
    res = run_bass_kernel_spmd(nc, in_maps, core_ids=list(range(NC)), trace=trace)
    kernel.last_exec_time_ns = res.exec_time_ns
    kernel.last_result = res
    y = np.zeros((G, 1), np.float32)
    for c in range(NC):
        og = res.results[c]["out_g"]
        cg = asm["core_graphs"][c]
        y[cg] = og[:len(cg)]
    return y



# revision 16
# speedup vs baseline: 1.1973x; 1.1973x over previous
"""GNN message-passing (PNA-style) Trainium2 Bass kernel, 8-core SPMD.

Self-contained: hardcodes problem shapes. kernel(**inputs) -> [4000, 1] f32.
"""
import sys
sys.path.insert(0, "/opt/trn_rl_repo")
import numpy as np
import ml_dtypes

import concourse.bass as bass
import concourse.bacc as bacc
import concourse.tile as tile
from concourse import mybir
from concourse.bass_utils import run_bass_kernel_spmd
from concourse.masks import make_identity

fp32 = mybir.dt.float32
bf16 = mybir.dt.bfloat16
i32 = mybir.dt.int32
BF = ml_dtypes.bfloat16
AF = mybir.ActivationFunctionType
OP = mybir.AluOpType

# problem constants
N, E, G, D = 100000, 400000, 4000, 70
NC = 8
ATOM_DIMS = np.array([119, 5, 12, 12, 10, 6, 6, 2, 2])
ATOM_OFFSETS = np.concatenate([[0], np.cumsum(ATOM_DIMS)[:-1]]).astype(np.int64)
DEG_HIST = np.array([0.0, 100.0, 400.0, 300.0, 200.0])
_bins = np.arange(len(DEG_HIST), dtype=np.float64)
AVG_LOG = float((np.log(_bins + 1.0) * DEG_HIST).sum() / DEG_HIST.sum())
BN_EPS = 1e-5
STD_EPS = 1e-5
P = 128
BT = 4          # tiles per processing block
EA_PAD = -1e9   # ea value for padding slots (message -> 0 after relu)
BIG = 1e9


def _insert_axis(ap_obj, pos, count):
    """Insert a broadcast axis [step=0, count] at free-dim position pos (0-based
    counting partition dim as index 0 of .ap)."""
    lst = [list(x) for x in ap_obj.ap]
    lst = lst[:pos] + [[0, count]] + lst[pos:]
    return bass.AP(ap_obj.tensor, ap_obj.offset, lst)


def _prep(x, edge_index, edge_attr, batch, atom_emb):
    """Host-side graph preprocessing. Returns cfg + per-core arrays."""
    src = np.asarray(edge_index[0], np.int64)
    dst = np.asarray(edge_index[1], np.int64)
    batch = np.asarray(batch, np.int64)
    ea = np.asarray(edge_attr, np.float32)

    deg = np.bincount(dst, minlength=N)
    # CSR by dst
    eorder = np.argsort(dst, kind="stable")
    rowptr = np.zeros(N + 1, np.int64)
    rowptr[1:] = np.cumsum(deg)

    # graph-aligned core node ranges
    gcnt = np.bincount(batch, minlength=G)
    gnode_start = np.zeros(G + 1, np.int64)
    gnode_start[1:] = np.cumsum(gcnt)
    core_gb = [0]
    for c in range(1, NC):
        target = c * N // NC
        gi = int(np.searchsorted(gnode_start, target))
        if gnode_start[gi] != target and gi > 0:
            gi = gi if abs(gnode_start[gi] - target) < abs(gnode_start[gi - 1] - target) else gi - 1
        core_gb.append(gi)
    core_gb.append(G)
    core_nodes = [(int(gnode_start[core_gb[c]]), int(gnode_start[core_gb[c + 1]])) for c in range(NC)]

    # degree groups: exact 1..8, tail >= 9 (padded to dtail)
    dmax = int(deg.max())
    exact_ds = [d for d in range(0, min(dmax, 8) + 1)]
    has_tail = dmax > 8
    dtail = dmax if has_tail else 0

    # per-core nodes per group
    core_group_nodes = []  # [c][g] -> node id array
    for c in range(NC):
        n0, n1 = core_nodes[c]
        nd = deg[n0:n1]
        groups = []
        for d in exact_ds:
            groups.append(np.nonzero(nd == d)[0] + n0)
        if has_tail:
            groups.append(np.nonzero(nd >= 9)[0] + n0)
        core_group_nodes.append(groups)

    ngroups = len(exact_ds) + (1 if has_tail else 0)
    dvals = exact_ds + ([dtail] if has_tail else [])
    NT_g = [max((len(core_group_nodes[c][g]) + P - 1) // P for c in range(NC)) for g in range(ngroups)]
    NT = sum(NT_g)
    NB = NT * P
    NPAD = NC * NB

    # proc order per core: group-major, node arrays padded with -1
    proc = np.full((NC, NB), -1, np.int64)
    tile_d = np.zeros(NT, np.int64)  # degree of each tile (common)
    ti = 0
    goff = []
    for g in range(ngroups):
        goff.append(ti)
        for t in range(NT_g[g]):
            tile_d[ti] = dvals[g]
            ti += 1
    for c in range(NC):
        for g in range(ngroups):
            nodes = core_group_nodes[c][g]
            off = goff[g] * P
            proc[c, off:off + len(nodes)] = nodes

    # pos_of_node (per core, global padded position)
    gpos_of_node = np.full(N, -1, np.int64)
    for c in range(NC):
        mask = proc[c] >= 0
        gpos_of_node[proc[c][mask]] = c * NB + np.nonzero(mask)[0]
    assert (gpos_of_node >= 0).sum() == N

    # blocks: per group, tiles chunked by BT
    blocks = []  # (g, d, t0, nb) with t0 global tile index
    for g in range(ngroups):
        d = dvals[g]
        for b0 in range(0, NT_g[g], BT):
            nb = min(BT, NT_g[g] - b0)
            blocks.append((g, d, goff[g] + b0, nb))

    # gather columns + ea layout + bigpad
    ncols = sum(d * nb for (_, d, _, nb) in blocks if d > 0)
    srcidx = np.full((NC, P, max(ncols, 1)), NPAD, np.int32)
    ea2d = np.full((NC, P, max(ncols, 1) * D), EA_PAD, np.float32)
    bigpad_cols = sum(dtail * nb for (g, d, _, nb) in blocks if has_tail and g == ngroups - 1)
    bigpad = np.zeros((NC, P, max(bigpad_cols, 1)), np.float32)

    col = 0
    bcol = 0
    ea_offsets = []  # block -> (g, d, t0, nb, col, bcol)
    for (g, d, t0, nb) in blocks:
        if d == 0:
            continue
        ea_offsets.append((g, d, t0, nb, col, 0, bcol))
        for c in range(NC):
            for i in range(nb):
                t = t0 + i
                for p in range(P):
                    node = proc[c, t * P + p]
                    if node < 0:
                        continue
                    nd = int(deg[node])
                    use = min(nd, d)
                    eids = eorder[rowptr[node]:rowptr[node] + use]
                    c0 = (col + i * d) * D
                    ea2d[c, p, c0:c0 + use * D] = ea[eids].ravel()
                    srcidx[c, p, col + i * d:col + i * d + use] = gpos_of_node[src[eids]]
                    if g == ngroups - 1 and has_tail and nd < d:
                        bigpad[c, p, bcol + i * d + nd:bcol + (i + 1) * d] = BIG
        col += nb * d
        if g == ngroups - 1 and has_tail:
            bcol += nb * d
    NIDX = col

    # split point for chunked allgather: block boundary at ~72% of gather cols
    cum = 0
    Tsplit = ea_offsets[-1][2] + ea_offsets[-1][3] if ea_offsets else NT
    for (g, d, t0, nb, bcol_, eoff_, bp_) in ea_offsets:
        cum += nb * d
        if cum >= 0.72 * NIDX:
            Tsplit = t0 + nb
            break
    H = Tsplit * P
    # gpos2: split-allgather layout: pos<H -> c*H+pos ; else 8H + c*(NB-H) + (pos-H)
    v = srcidx.astype(np.int64)
    cc_ = v // NB
    pp_ = v - cc_ * NB
    lo = v < NPAD
    g2 = np.where(pp_ < H, cc_ * H + pp_, NC * H + cc_ * (NB - H) + (pp_ - H))
    srcidx2 = np.where(lo, g2, NPAD).astype(np.int32)

    # statics in proc order [NC, P, 3, NT] (partition-major for one-shot load)
    statn = np.zeros((NC, P, 3, NT), np.float32)
    for c in range(NC):
        nodes = proc[c]
        dd = np.where(nodes >= 0, deg[np.clip(nodes, 0, N - 1)], 0).astype(np.float64)
        dsafe = np.maximum(dd, 1.0)
        amp = np.log(dsafe + 1.0) / AVG_LOG
        statn[c, :, 0] = (1.0 / dsafe).astype(np.float32).reshape(NT, P).T
        statn[c, :, 1] = amp.astype(np.float32).reshape(NT, P).T
        statn[c, :, 2] = (1.0 / amp).astype(np.float32).reshape(NT, P).T

    # h0 (AtomEncoder) host-side, proc order
    xl = np.asarray(x, np.int64)
    emb = np.asarray(atom_emb, np.float32)
    h0_all = emb[xl + ATOM_OFFSETS[None, :]].sum(axis=1)  # [N, D]
    h0_own = np.zeros((NC, NB, D), np.float32)
    for c in range(NC):
        mask = proc[c] >= 0
        h0_own[c][mask] = h0_all[proc[c][mask]]

    # pooling: per core graphs sorted by size desc, common tile grid
    core_graphs = []
    for c in range(NC):
        gids = np.arange(core_gb[c], core_gb[c + 1])
        order = np.argsort(-gcnt[gids], kind="stable")
        core_graphs.append(gids[order])
    NGT = max((len(cg) + P - 1) // P for cg in core_graphs)
    KG_t = []
    for t in range(NGT):
        m = 1
        for c in range(NC):
            cg = core_graphs[c]
            if t * P < len(cg):
                m = max(m, int(gcnt[cg[t * P]]))
        KG_t.append(m)
    npoolcols = sum(KG_t)
    poolidx = np.full((NC, P, npoolcols), NB, np.int32)
    ginv = np.ones((NC, NGT, P), np.float32)
    pc = 0
    pool_cols = []
    for t in range(NGT):
        pool_cols.append(pc)
        for c in range(NC):
            cg = core_graphs[c]
            for p in range(P):
                if t * P + p >= len(cg):
                    continue
                gid = cg[t * P + p]
                sz = int(gcnt[gid])
                ginv[c, t, p] = 1.0 / max(sz, 1)
                if sz > 0:
                    nids = np.arange(gnode_start[gid], gnode_start[gid] + sz)
                    pos = gpos_of_node[nids] - (gpos_of_node[nids] // NB) * NB
                    # nodes of this graph belong to core c by construction
                    poolidx[c, p, pc:pc + sz] = pos.astype(np.int32)
        pc += KG_t[t]

    cfg = dict(NB=NB, NT=NT, NPAD=NPAD, NIDX=NIDX, Tsplit=Tsplit, H=H,
               blocks=blocks, ea_offsets=ea_offsets, dvals=dvals, goff=goff,
               NT_g=NT_g, ngroups=ngroups, has_tail=has_tail, dtail=dtail,
               NGT=NGT, KG_t=KG_t, pool_cols=pool_cols, npoolcols=npoolcols,
               bigpad_cols=max(bigpad_cols, 1))
    h0_full = np.zeros((NPAD + 1, D), np.float32)
    for c in range(NC):
        h0_full[c * NB:(c + 1) * NB] = h0_own[c]
    arrays = dict(srcidx=srcidx, srcidx2=srcidx2, ea2d=ea2d, statn=statn,
                  h0_own=h0_own, h0_full=h0_full, poolidx=poolidx, ginv=ginv, bigpad=bigpad)
    asm = dict(core_graphs=core_graphs, core_gb=core_gb)
    return cfg, arrays, asm


def _prep_weights(post_w, post_b, bn_gamma, bn_beta, mlp_w1, mlp_b1, mlp_w2, mlp_b2, mlp_w3, mlp_b3):
    post_w = np.asarray(post_w, np.float32)   # [4, 840, 70]
    post_b = np.asarray(post_b, np.float32)   # [4, 70]
    bn_gamma = np.asarray(bn_gamma, np.float32)
    bn_beta = np.asarray(bn_beta, np.float32)
    inv_std_bn = np.float32(1.0 / np.sqrt(1.0 + BN_EPS))
    # wch [4, 3 chunks, 128, 210]: chunk k rows k*128..; cols = A|B|C (70 each)
    wch = np.zeros((4, 3, P, 210), np.float32)
    for l in range(4):
        for ch in range(3):
            r0, r1 = ch * 128, min((ch + 1) * 128, 280)
            rows = r1 - r0
            for s in range(3):  # A, B, C weight sets at rows s*280
                wch[l, ch, :rows, s * 70:(s + 1) * 70] = post_w[l, s * 280 + r0:s * 280 + r1, :]
    Grep = (bn_gamma * inv_std_bn)                     # [4, 70]
    B2 = post_b * Grep + bn_beta                       # [4, 70]
    w1 = np.asarray(mlp_w1, np.float32)                # [70, 35]
    w2 = np.asarray(mlp_w2, np.float32)                # [35, 17]
    w3 = np.asarray(mlp_w3, np.float32)                # [17, 1]
    b1 = np.asarray(mlp_b1, np.float32)
    b2 = np.asarray(mlp_b2, np.float32)
    b3 = np.asarray(mlp_b3, np.float32)
    # repsb [128, 4*70 + 4*70] bf16 (Grep | B2); repsf [128, 53] fp32 (b1|b2|b3)
    repsb = np.concatenate([Grep.ravel(), B2.ravel()]).astype(np.float32)
    repsb = np.broadcast_to(repsb, (P, repsb.size)).copy()
    repsf = np.concatenate([b1, b2, b3]).astype(np.float32)
    repsf = np.broadcast_to(repsf, (P, repsf.size)).copy()
    return dict(wch=wch, repsb=repsb, repsf=repsf, w1=w1, w2=w2, w3=w3)


def _build(cfg):
    NB, NT, NPAD, NIDX = cfg["NB"], cfg["NT"], cfg["NPAD"], cfg["NIDX"]
    NGT, npoolcols = cfg["NGT"], cfg["npoolcols"]
    NREPB = 4 * 70 + 4 * 70
    NREPF = 35 + 17 + 1

    nc = bacc.Bacc("TRN2", target_bir_lowering=False, debug=False, num_devices=NC)
    # inputs
    h0_own = nc.dram_tensor("h0_own", [NB, D], bf16, kind="ExternalInput").ap()
    h0_full = nc.dram_tensor("h0_full", [NPAD + 1, D], bf16, kind="ExternalInput").ap()
    srcidx2 = nc.dram_tensor("srcidx2", [P, max(NIDX, 1)], i32, kind="ExternalInput").ap()
    ea2d = nc.dram_tensor("ea2d", [P, max(NIDX, 1) * D], bf16, kind="ExternalInput").ap()
    srcidx = nc.dram_tensor("srcidx", [P, max(NIDX, 1)], i32, kind="ExternalInput").ap()
    statn = nc.dram_tensor("statn", [P, 3 * NT], bf16, kind="ExternalInput").ap()
    bigpad_t = nc.dram_tensor("bigpad", [P, cfg["bigpad_cols"]], bf16, kind="ExternalInput").ap()
    poolidx = nc.dram_tensor("poolidx", [P, npoolcols], i32, kind="ExternalInput").ap()
    ginv = nc.dram_tensor("ginv", [NGT, P], fp32, kind="ExternalInput").ap()
    wch = nc.dram_tensor("wch", [4, 3, P, 210], bf16, kind="ExternalInput").ap()
    repsb = nc.dram_tensor("repsb", [P, NREPB], bf16, kind="ExternalInput").ap()
    repsf = nc.dram_tensor("repsf", [P, NREPF], fp32, kind="ExternalInput").ap()
    w1 = nc.dram_tensor("w1", [D, 35], fp32, kind="ExternalInput").ap()
    w2 = nc.dram_tensor("w2", [35, 17], fp32, kind="ExternalInput").ap()
    w3 = nc.dram_tensor("w3", [17, 1], fp32, kind="ExternalInput").ap()
    out_g = nc.dram_tensor("out_g", [NGT * P, 1], fp32, kind="ExternalOutput").ap()

    # internal DRAM
    h_own = [nc.dram_tensor(f"h_own{l}", [NB + 1, D], bf16) for l in range(5)]
    hbuf = [None] + [nc.dram_tensor(f"hbuf{l}", [NPAD + 1, D], bf16, addr_space="Shared")
                     for l in range(1, 4)]

    # persistent SBUF
    idx_sb = nc.alloc_sbuf_tensor("idx_sb", [P, max(NIDX, 1)], i32).ap()
    idx2_sb = nc.alloc_sbuf_tensor("idx2_sb", [P, max(NIDX, 1)], i32).ap()
    statn_sb = nc.alloc_sbuf_tensor("statn_sb", [P, 3 * NT], bf16).ap()
    ea_sb = nc.alloc_sbuf_tensor("ea_sb", [P, max(NIDX, 1) * D], bf16).ap()
    hA_sb = nc.alloc_sbuf_tensor("hA_sb", [P, NT * D], bf16).ap()
    hB_sb = nc.alloc_sbuf_tensor("hB_sb", [P, NT * D], bf16).ap()
    pidx_sb = nc.alloc_sbuf_tensor("pidx_sb", [P, npoolcols], i32).ap()
    wch_sb = nc.alloc_sbuf_tensor("wch_sb", [P, 4 * 3 * 210], bf16).ap()
    repsb_sb = nc.alloc_sbuf_tensor("repsb_sb", [P, NREPB], bf16).ap()
    repsf_sb = nc.alloc_sbuf_tensor("repsf_sb", [P, NREPF], fp32).ap()
    w1_sb = nc.alloc_sbuf_tensor("w1_sb", [D, 35], fp32).ap()
    w2_sb = nc.alloc_sbuf_tensor("w2_sb", [35, 17], fp32).ap()
    w3_sb = nc.alloc_sbuf_tensor("w3_sb", [17, 1], fp32).ap()
    ident = nc.alloc_sbuf_tensor("ident", [P, P], fp32).ap()
    identb = nc.alloc_sbuf_tensor("identb", [P, P], bf16).ap()
    epsb = nc.alloc_sbuf_tensor("epsb", [P, 1], bf16).ap()
    zrow = nc.alloc_sbuf_tensor("zrow", [1, D], bf16).ap()

    cc_sems = {(l, h): nc.alloc_semaphore(name=f"ccs{l}_{h}") for l in range(1, 4) for h in range(2)}

    # ---- segment 0: load persistents, init dummies, stage h0 ----
    with tile.TileContext(nc) as tc:
        with tc.tile_pool(name="s0", bufs=2) as pool:
            nc.sync.dma_start(out=idx_sb[:, :], in_=srcidx[:, :])
            nc.sync.dma_start(out=idx2_sb[:, :], in_=srcidx2[:, :])
            nc.sync.dma_start(out=statn_sb[:, :], in_=statn[:, :])
            nc.sync.dma_start(out=ea_sb[:, :], in_=ea2d[:, :])
            nc.sync.dma_start(out=hA_sb[:].rearrange("p (t f) -> p t f", t=NT),
                              in_=h0_own[:, :].rearrange("(t p) f -> p t f", p=P))
            nc.sync.dma_start(out=pidx_sb[:, :], in_=poolidx[:, :])
            nc.sync.dma_start(out=wch_sb[:].rearrange("p (l c f) -> p l c f", l=4, c=3),
                              in_=wch.rearrange("l c p f -> p l c f"))
            nc.sync.dma_start(out=repsb_sb[:, :], in_=repsb[:, :])
            nc.sync.dma_start(out=repsf_sb[:, :], in_=repsf[:, :])
            nc.sync.dma_start(out=w1_sb[:, :], in_=w1[:, :])
            nc.sync.dma_start(out=w2_sb[:, :], in_=w2[:, :])
            nc.sync.dma_start(out=w3_sb[:, :], in_=w3[:, :])
            make_identity(nc, ident[:])
            make_identity(nc, identb[:])
            nc.vector.memset(epsb[:], STD_EPS)
            nc.vector.memset(zrow[:], 0.0)
            for l in range(1, 4):
                nc.sync.dma_start(out=hbuf[l].ap()[NPAD:NPAD + 1, :], in_=zrow[:])
            nc.sync.dma_start(out=h_own[4].ap()[NB:NB + 1, :], in_=zrow[:])

    H = cfg["H"]
    Tsplit = cfg["Tsplit"]

    def do_cc(l, part):
        if part == 0:
            ins_ap = h_own[l].ap()[0:H, :].opt()
            outs_ap = hbuf[l].ap()[0:NC * H, :].opt()
        else:
            ins_ap = h_own[l].ap()[H:NB, :].opt()
            outs_ap = hbuf[l].ap()[NC * H:NPAD, :].opt()
        nc.gpsimd.collective_compute(
            "AllGather", OP.bypass,
            replica_groups=[list(range(NC))],
            ins=[ins_ap], outs=[outs_ap],
        ).then_inc(cc_sems[(l, part)])

    def hbufs(l):
        """(hcur, hnext) SBUF node-major [P, NT, D] views for layer l."""
        cur = hA_sb if l % 2 == 1 else hB_sb
        nxt = hB_sb if l % 2 == 1 else hA_sb
        return (cur[:].rearrange("p (t f) -> p t f", t=NT),
                nxt[:].rearrange("p (t f) -> p t f", t=NT))

    def emit_msg_block(blk, l, hprev_full, idxtile, pool, spool, psp, gpool):
        (g, d, t0, nb, col, _eoff, bcol) = blk
        X = nb * d * D
        hcur, hnxt = hbufs(l)
        gsrc = gpool.tile([P, X], bf16, tag="gsrc")
        for k in range(nb * d):
            nc.gpsimd.indirect_dma_start(
                out=gsrc[:, k * D:(k + 1) * D],
                out_offset=None,
                in_=hprev_full[:, :],
                in_offset=bass.IndirectOffsetOnAxis(ap=idxtile[:, col + k:col + k + 1], axis=0),
            )
        gblk = gpool.tile([P, X], bf16, tag="gblk")
        nc.vector.tensor_tensor(out=gblk[:], in0=gsrc[:], in1=ea_sb[:, col * D:(col + nb * d) * D], op=OP.add)
        hdst_b = _insert_axis(hcur[:, t0:t0 + nb, :], 2, d)
        g3 = gblk[:].rearrange("p (t j f) -> p t j f", t=nb, j=d)
        nc.vector.tensor_tensor(out=g3, in0=g3, in1=hdst_b, op=OP.add)
        nc.scalar.activation(out=gblk[:], in_=gblk[:], func=AF.Relu)

        agg = spool.tile([P, nb * 280], bf16, tag="agg")
        a3 = agg[:].rearrange("p (t f) -> p t f", t=nb)
        mn_out = a3[:, :, 70:140]
        mx_out = a3[:, :, 140:210]
        is_tail = cfg["has_tail"] and g == cfg["ngroups"] - 1
        if d == 1:
            nc.vector.tensor_copy(out=mn_out, in_=gblk[:].rearrange("p (t f) -> p t f", t=nb))
            nc.vector.tensor_copy(out=mx_out, in_=gblk[:].rearrange("p (t f) -> p t f", t=nb))
            s_src = gblk[:].rearrange("p (t f) -> p t f", t=nb)
            nc.vector.tensor_copy(out=a3[:, :, 0:70], in_=s_src)
            nc.scalar.activation(out=gblk[:], in_=gblk[:], func=AF.Square)
            s2_fin = gblk[:].rearrange("p (t f) -> p t f", t=nb)
            s_fin = a3[:, :, 0:70]
        else:
            g4 = gblk[:].rearrange("p (t j f) -> p t j f", t=nb, j=d)
            if is_tail:
                bp = pool.tile([P, nb * d], bf16, tag="bp")
                nc.sync.dma_start(out=bp[:], in_=bigpad_t[:, bcol:bcol + nb * d])
                mfm = spool.tile([P, X], bf16, tag="mfm")
                m4 = mfm[:].rearrange("p (t j f) -> p t j f", t=nb, j=d)
                bp_b = _insert_axis(bp[:].rearrange("p (t j) -> p t j", t=nb), 3, D)
                nc.vector.tensor_tensor(out=m4, in0=g4, in1=bp_b, op=OP.add)
                _fold_minmax(nc, spool, m4, d, nb, mn_out, OP.min, "mnscr")
            else:
                _fold_minmax(nc, spool, g4, d, nb, mn_out, OP.min, "mnscr")
            _fold_minmax(nc, spool, g4, d, nb, mx_out, OP.max, "mxscr")
            _fold_sum(nc, spool, g4, d, nb, a3[:, :, 0:70], "sscr")
            nc.scalar.activation(out=gblk[:], in_=gblk[:], func=AF.Square)
            s2t = spool.tile([P, nb * D], bf16, tag="s2t")
            _fold_sum(nc, spool, g4, d, nb, s2t[:].rearrange("p (t f) -> p t f", t=nb), "s2scr")
            s2_fin = s2t[:].rearrange("p (t f) -> p t f", t=nb)
            s_fin = a3[:, :, 0:70]
        _stage2(nc, pool, spool, psp, cfg, statn_sb, a3, s_fin, s2_fin,
                t0, nb, l, wch_sb, repsb_sb, identb, epsb, hcur, hnxt, h_own[l].ap(), d)

    def emit_d0_block(blk, l, pool, spool, psp):
        (g, d, t0, nb) = blk
        hcur, hnxt = hbufs(l)
        agg = spool.tile([P, nb * 280], bf16, tag="agg")
        nc.vector.memset(agg[:], 0.0)
        a3 = agg[:].rearrange("p (t f) -> p t f", t=nb)
        _stage2(nc, pool, spool, psp, cfg, statn_sb, a3, a3[:, :, 0:70], a3[:, :, 0:70],
                t0, nb, l, wch_sb, repsb_sb, identb, epsb, hcur, hnxt, h_own[l].ap(), d)

    easA = [b for b in cfg["ea_offsets"] if b[2] < Tsplit]
    easB = [b for b in cfg["ea_offsets"] if b[2] >= Tsplit]
    d0sA = [b for b in cfg["blocks"] if b[1] == 0 and b[2] < Tsplit]
    d0sB = [b for b in cfg["blocks"] if b[1] == 0 and b[2] >= Tsplit]
    for b in easA + [x + (0, 0, 0) for x in d0sA]:
        assert b[2] + b[3] <= Tsplit, b

    # ---- layers ----
    for l in range(1, 5):
        hprev_full = h0_full if l == 1 else hbuf[l - 1].ap()
        idxtile = idx_sb if l == 1 else idx2_sb
        if l >= 2 and l - 1 <= 3:
            nc.gpsimd.wait_ge(cc_sems[(l - 1, 0)], 1)
            nc.gpsimd.wait_ge(cc_sems[(l - 1, 1)], 1)
        if l < 4:
            with tile.TileContext(nc) as tc:
                with tc.tile_pool(name=f"L{l}a", bufs=2) as pool, \
                     tc.tile_pool(name=f"Lg{l}a", bufs=3) as gpool, \
                     tc.tile_pool(name=f"Ls{l}a", bufs=1) as spool, \
                     tc.tile_pool(name=f"Lp{l}a", bufs=2, space="PSUM") as psp:
                    for blk in easA:
                        emit_msg_block(blk, l, hprev_full, idxtile, pool, spool, psp, gpool)
                    for blk in d0sA:
                        emit_d0_block(blk, l, pool, spool, psp)
            do_cc(l, 0)
            with tile.TileContext(nc) as tc:
                with tc.tile_pool(name=f"L{l}b", bufs=2) as pool, \
                     tc.tile_pool(name=f"Lg{l}b", bufs=3) as gpool, \
                     tc.tile_pool(name=f"Ls{l}b", bufs=1) as spool, \
                     tc.tile_pool(name=f"Lp{l}b", bufs=2, space="PSUM") as psp:
                    for blk in easB:
                        emit_msg_block(blk, l, hprev_full, idxtile, pool, spool, psp, gpool)
                    for blk in d0sB:
                        emit_d0_block(blk, l, pool, spool, psp)
            do_cc(l, 1)
        else:
            with tile.TileContext(nc) as tc:
                with tc.tile_pool(name=f"L{l}", bufs=2) as pool, \
                     tc.tile_pool(name=f"Lg{l}", bufs=3) as gpool, \
                     tc.tile_pool(name=f"Ls{l}", bufs=1) as spool, \
                     tc.tile_pool(name=f"Lp{l}", bufs=2, space="PSUM") as psp:
                    for blk in easA + easB:
                        emit_msg_block(blk, l, hprev_full, idxtile, pool, spool, psp, gpool)
                    for blk in d0sA + d0sB:
                        emit_d0_block(blk, l, pool, spool, psp)
                    _pooling(nc, pool, spool, psp, cfg, pidx_sb, ginv, h_own[4].ap(),
                             w1_sb, w2_sb, w3_sb, repsf_sb, ident, identb, out_g)

    nc.compile()
    return nc


def _fold_minmax(nc, spool, g4, d, nb, out_slice, op, tag):
    k = d
    cur = g4
    first = True
    while k > 1:
        h = (k + 1) // 2
        if k == 2:
            nc.vector.tensor_tensor(
                out=out_slice,
                in0=cur[:, :, 0:1].rearrange("p t j f -> p t (j f)"),
                in1=cur[:, :, 1:2].rearrange("p t j f -> p t (j f)"), op=op)
            return
        if first:
            scr = spool.tile([g4.shape[0], nb * h * 70], bf16, tag=tag)
            scr3 = scr[:].rearrange("p (t j f) -> p t j f", t=nb, j=h)
            nc.vector.tensor_tensor(out=scr3[:, :, 0:h], in0=cur[:, :, 0:h], in1=cur[:, :, k - h:k], op=op)
            cur = scr3
            first = False
        else:
            nc.vector.tensor_tensor(out=cur[:, :, 0:h], in0=cur[:, :, 0:h], in1=cur[:, :, k - h:k], op=op)
        k = h


def _fold_sum(nc, spool, g4, d, nb, out_slice, tag):
    """sum over j; out_slice [P, nb, 70]."""
    k = d
    cur = g4
    first = True
    while k > 1:
        h = k // 2
        rem = k - h
        if k == 2:
            nc.vector.tensor_tensor(
                out=out_slice,
                in0=cur[:, :, 0:1].rearrange("p t j f -> p t (j f)"),
                in1=cur[:, :, 1:2].rearrange("p t j f -> p t (j f)"), op=OP.add)
            return
        if first:
            scr = spool.tile([g4.shape[0], nb * rem * 70], bf16, tag=tag)
            scr3 = scr[:].rearrange("p (t j f) -> p t j f", t=nb, j=rem)
            nc.vector.tensor_tensor(out=scr3[:, :, 0:h], in0=cur[:, :, 0:h], in1=cur[:, :, k - h:k], op=OP.add)
            if k % 2 == 1:
                nc.vector.tensor_copy(out=scr3[:, :, h:h + 1], in_=cur[:, :, h:h + 1])
            cur = scr3
            first = False
        else:
            nc.vector.tensor_tensor(out=cur[:, :, 0:h], in0=cur[:, :, 0:h], in1=cur[:, :, k - h:k], op=OP.add)
        k = rem


def _stage2(nc, pool, spool, psp, cfg, statn_sb, a3, s_fin, s2_fin,
            t0, nb, l, wch_sb, repsb_sb, identb, epsb, hcur, hnxt, hout, d):
    P_ = 128
    NT = cfg["NT"]
    invc_b = _insert_axis(statn_sb[:, 0 * NT + t0:0 * NT + t0 + nb], 2, 70)
    amp_b = _insert_axis(statn_sb[:, 1 * NT + t0:1 * NT + t0 + nb], 2, 70)
    iamp_b = _insert_axis(statn_sb[:, 2 * NT + t0:2 * NT + t0 + nb], 2, 70)

    if d > 0:
        # mean
        nc.vector.tensor_tensor(out=a3[:, :, 0:70], in0=s_fin, in1=invc_b, op=OP.mult)
        # var/std
        u = spool.tile([P_, nb * 70], bf16, tag="u")
        u3 = u[:].rearrange("p (t f) -> p t f", t=nb)
        nc.vector.tensor_tensor(out=u3, in0=s2_fin, in1=invc_b, op=OP.mult)
        v = spool.tile([P_, nb * 70], bf16, tag="v")
        v3 = v[:].rearrange("p (t f) -> p t f", t=nb)
        nc.vector.tensor_tensor(out=v3, in0=a3[:, :, 0:70], in1=a3[:, :, 0:70], op=OP.mult)
        nc.vector.tensor_tensor(out=u3, in0=u3, in1=v3, op=OP.subtract)
        nc.scalar.activation(out=u[:], in_=u[:], func=AF.Relu)
        nc.scalar.activation(out=a3[:, :, 210:280], in_=u3, func=AF.Sqrt, bias=epsb[:])
    else:
        # all-zero aggregates; std = sqrt(eps)
        nc.scalar.activation(out=a3[:, :, 210:280], in_=a3[:, :, 0:70], func=AF.Sqrt, bias=epsb[:])

    # post matmul per tile
    sabc = spool.tile([P_, nb * 210], bf16, tag="sabc")
    for i in range(nb):
        aggT = pool.tile([P_, P_], bf16, tag="aggT")
        psmm = psp.tile([P_, 210], fp32, space="PSUM", tag="psmm")
        for ch in range(3):
            rows = 128 if ch < 2 else 24
            psT = psp.tile([P_, P_], bf16, space="PSUM", tag="psT")
            nc.tensor.transpose(out=psT[:rows, :], in_=a3[:, i:i + 1, ch * 128:ch * 128 + rows].rearrange("p t f -> p (t f)"),
                                identity=identb[:])
            nc.vector.tensor_copy(out=aggT[:rows, :], in_=psT[:rows, :])
            nc.tensor.matmul(out=psmm[:, :], lhsT=aggT[:rows, :],
                             rhs=wch_sb[:rows, (l - 1) * 630 + ch * 210:(l - 1) * 630 + (ch + 1) * 210],
                             start=(ch == 0), stop=(ch == 2))
        nc.vector.tensor_copy(out=sabc[:, i * 210:(i + 1) * 210], in_=psmm[:, :])

    sA = sabc[:].rearrange("p (t f) -> p t f", t=nb)[:, :, 0:70]
    sB = sabc[:].rearrange("p (t f) -> p t f", t=nb)[:, :, 70:140]
    sC = sabc[:].rearrange("p (t f) -> p t f", t=nb)[:, :, 140:210]
    hn = pool.tile([P_, nb * 70], bf16, tag="hn")
    hn3 = hn[:].rearrange("p (t f) -> p t f", t=nb)
    tmp = pool.tile([P_, nb * 70], bf16, tag="tmp")
    tmp3 = tmp[:].rearrange("p (t f) -> p t f", t=nb)
    nc.vector.tensor_tensor(out=hn3, in0=sB, in1=amp_b, op=OP.mult)
    nc.vector.tensor_tensor(out=tmp3, in0=sC, in1=iamp_b, op=OP.mult)
    nc.vector.tensor_tensor(out=hn3, in0=hn3, in1=tmp3, op=OP.add)
    nc.vector.tensor_tensor(out=hn3, in0=hn3, in1=sA, op=OP.add)
    # BN affine + relu
    Grep_b = _insert_axis(repsb_sb[:, (l - 1) * 70:l * 70], 1, nb)
    B2_b = _insert_axis(repsb_sb[:, 280 + (l - 1) * 70:280 + l * 70], 1, nb)
    nc.vector.tensor_tensor(out=hn3, in0=hn3, in1=Grep_b, op=OP.mult)
    nc.vector.tensor_tensor(out=hn3, in0=hn3, in1=B2_b, op=OP.add)
    nc.scalar.activation(out=hn[:], in_=hn[:], func=AF.Relu)
    # residual into SBUF hnext, then stream to DRAM for the collective
    nc.vector.tensor_tensor(out=hnxt[:, t0:t0 + nb, :], in0=hn3, in1=hcur[:, t0:t0 + nb, :], op=OP.add)
    nc.sync.dma_start(out=hout[t0 * P_:(t0 + nb) * P_, :].rearrange("(t p) f -> p t f", p=P_),
                      in_=hnxt[:, t0:t0 + nb, :])


def _pooling(nc, pool, spool, psp, cfg, pidx_sb, ginv, h4, w1_sb, w2_sb, w3_sb,
             repsf_sb, ident, identb, out_g):
    P_ = 128
    fp = fp32
    for t in range(cfg["NGT"]):
        KG = cfg["KG_t"][t]
        pc = cfg["pool_cols"][t]
        pg = pool.tile([P_, KG * D], bf16, tag="pg")
        for j in range(KG):
            nc.gpsimd.indirect_dma_start(
                out=pg[:, j * D:(j + 1) * D], out_offset=None,
                in_=h4[:, :],
                in_offset=bass.IndirectOffsetOnAxis(ap=pidx_sb[:, pc + j:pc + j + 1], axis=0))
        gsum = pool.tile([P_, D], fp, tag="gsum")
        nc.vector.tensor_reduce(out=gsum[:], in_=pg[:].rearrange("p (k f) -> p f k", k=KG),
                                op=OP.add, axis=mybir.AxisListType.X)
        gv = pool.tile([P_, 1], fp, tag="gv")
        nc.sync.dma_start(out=gv[:], in_=ginv[t:t + 1, :].rearrange("o p -> p o"))
        nc.vector.tensor_scalar_mul(gsum[:], gsum[:], gv[:])
        # mlp
        psT = psp.tile([P_, P_], fp, space="PSUM", tag="psT")
        nc.tensor.transpose(out=psT[:D, :], in_=gsum[:], identity=ident[:])
        gT = pool.tile([D, P_], fp, tag="gT")
        nc.vector.tensor_copy(out=gT[:], in_=psT[:D, :])
        ps1 = psp.tile([P_, 35], fp, space="PSUM", tag="psmm")
        nc.tensor.matmul(out=ps1[:], lhsT=gT[:], rhs=w1_sb[:, :], start=True, stop=True)
        y1 = pool.tile([P_, 35], fp, tag="y1")
        nc.vector.tensor_tensor(out=y1[:], in0=ps1[:], in1=repsf_sb[:, 0:35], op=OP.add)
        nc.scalar.activation(out=y1[:], in_=y1[:], func=AF.Relu)
        psT2 = psp.tile([P_, P_], fp, space="PSUM", tag="psT")
        nc.tensor.transpose(out=psT2[:35, :], in_=y1[:], identity=ident[:])
        y1T = pool.tile([35, P_], fp, tag="y1T")
        nc.vector.tensor_copy(out=y1T[:], in_=psT2[:35, :])
        ps2 = psp.tile([P_, 17], fp, space="PSUM", tag="psmm")
        nc.tensor.matmul(out=ps2[:], lhsT=y1T[:], rhs=w2_sb[:, :], start=True, stop=True)
        y2 = pool.tile([P_, 17], fp, tag="y2")
        nc.vector.tensor_tensor(out=y2[:], in0=ps2[:], in1=repsf_sb[:, 35:52], op=OP.add)
        nc.scalar.activation(out=y2[:], in_=y2[:], func=AF.Relu)
        psT3 = psp.tile([P_, P_], fp, space="PSUM", tag="psT")
        nc.tensor.transpose(out=psT3[:17, :], in_=y2[:], identity=ident[:])
        y2T = pool.tile([17, P_], fp, tag="y2T")
        nc.vector.tensor_copy(out=y2T[:], in_=psT3[:17, :])
        ps3 = psp.tile([P_, 1], fp, space="PSUM", tag="psmm")
        nc.tensor.matmul(out=ps3[:], lhsT=y2T[:], rhs=w3_sb[:, :], start=True, stop=True)
        y3 = pool.tile([P_, 1], fp, tag="y3")
        nc.vector.tensor_tensor(out=y3[:], in0=ps3[:], in1=repsf_sb[:, 52:53], op=OP.add)
        nc.sync.dma_start(out=out_g[t * P_:(t + 1) * P_, :], in_=y3[:])


def kernel(x, edge_index, edge_attr, batch, atom_emb, post_w, post_b,
           bn_gamma, bn_beta, mlp_w1, mlp_b1, mlp_w2, mlp_b2, mlp_w3, mlp_b3):
    cfg, arrays, asm = _prep(x, edge_index, edge_attr, batch, atom_emb)
    wd = _prep_weights(post_w, post_b, bn_gamma, bn_beta, mlp_w1, mlp_b1,
                       mlp_w2, mlp_b2, mlp_w3, mlp_b3)
    nc = _build(cfg)

    in_maps = []
    for c in range(NC):
        in_maps.append({
            "h0_own": arrays["h0_own"][c].astype(BF),
            "ea2d": arrays["ea2d"][c].astype(BF),
            "srcidx": arrays["srcidx"][c],
            "srcidx2": arrays["srcidx2"][c],
            "h0_full": arrays["h0_full"].astype(BF),
            "statn": arrays["statn"][c].reshape(P, -1).astype(BF),
            "bigpad": arrays["bigpad"][c].astype(BF),
            "poolidx": arrays["poolidx"][c],
            "ginv": arrays["ginv"][c],
            "wch": wd["wch"].astype(BF),
            "repsb": wd["repsb"].astype(BF),
            "repsf": wd["repsf"],
            "w1": wd["w1"],
            "w2": wd["w2"],
            "w3": wd["w3"],
        })
    import os
    trace = os.environ.get("KERNEL_TRACE", "0") == "1"
    res = run_bass_kernel_spmd(nc, in_maps, core_ids=list(range(NC)), trace=trace)
    kernel.last_exec_time_ns = res.exec_time_ns
    kernel.last_result = res
    y = np.zeros((G, 1), np.float32)
    for c in range(NC):
        og = res.results[c]["out_g"]
        cg = asm["core_graphs"][c]
        y[cg] = og[:len(cg)]
    return y
